# revision 9
# baseline (speedup 1.0000x reference)
"""Trainium2 Bass kernel for nn_CrossCorrelationComputation.

corr[q,s,p,k] = sum_c Qn[q,c,p] * Sn[s,c,p+delta_k]
  Qn/Sn L2-normalized over c (=640); p over 14x14 spatial, k over 5x5 offsets
  (zero-padded); output (75, 25, 196, 25) fp32.

Strategy: shard spatial rows across 8 cores (6 cores x 2 rows, 2 cores x 1 row;
every core runs a uniform 28-position program, pad positions discarded on the
host).  Per core the full q=75 is the matmul stationary dim, contraction over
c in 5 chunks of 128 partitions, and the 5x5 unfold window is a strided AP
view into an x-padded support tile (no gather).  Normalization on device:
squares (ACT) -> cross-partition reduce via ones-matmul (PE) -> sqrt (ACT) ->
reciprocal (DVE); support scaled in place (DVE), query scale fused into the
PSUM->SBUF copy as a per-partition scalar.

Sync-wait discipline (hardware limits: fp32 Matmult and HWDGE DMA carry at
most ONE semaphore wait):
  - all data DMAs go through gpsimd SWDGE with num_swdge_queues=1 (one sem)
  - a tiny "absorb" matmul makes PE observe that sem once, early
  - matmul operand tiles then only need their last compute-engine writer
"""

import numpy as np

import concourse.bass as bass
import concourse.mybir as mybir
import concourse.tile as tile
from concourse import bacc
from concourse.bass_utils import run_bass_kernel_spmd

F32 = mybir.dt.float32

NQ, NS, C, H, W = 75, 25, 640, 14, 14
KK = 25                      # 5x5 offsets
P = 128                      # partitions
NCH = C // P                 # 5 c-chunks
XP = W + 4                   # x padded to 18
RT = 6                       # support tile rows: 2 + 2*2 halo
VR = 2                       # virtual rows per core
PCNT = VR * W                # 28 positions per core
NCORES = 8
ROW_BASE = [0, 2, 4, 6, 8, 10, 12, 13]   # first real row per core
ROW_CNT = [2, 2, 2, 2, 2, 2, 1, 1]

SP_COLS = NS * RT * XP       # 2700 padded support cols per chunk
Q_COLS = PCNT * NQ           # 2100 query cols per chunk
NBLK = 512

_NC_CACHE = {}


def _ceil_blocks(n, b):
    return [(i, min(b, n - i)) for i in range(0, n, b)]


def build_nc():
    nc = bacc.Bacc(trn_type="TRN2", num_swdge_queues=1)
    qin = nc.dram_tensor("qin", [P, NCH, PCNT, NQ], F32, kind="ExternalInput")
    sin = nc.dram_tensor("sin", [P, NCH, NS, RT, XP], F32, kind="ExternalInput")
    out = nc.dram_tensor("out", [NQ, NS, PCNT, KK], F32, kind="ExternalOutput")

    ones = nc.const_aps.tensor(1.0, (P, 1), F32)

    with tile.TileContext(nc) as tc:
        with (
            tc.tile_pool(name="big", bufs=1) as big,
            tc.tile_pool(name="sq", bufs=3) as sqp,
            tc.tile_pool(name="stage", bufs=2) as stp,
            tc.tile_pool(name="psn", bufs=2, space="PSUM") as psn,
            tc.tile_pool(name="psd", bufs=1, space="PSUM") as psd,
            tc.tile_pool(name="psa", bufs=2, space="PSUM") as psa,
            tc.tile_pool(name="psb", bufs=2, space="PSUM") as psb,
            tc.tile_pool(name="dram", bufs=1, space="DRAM") as dram,
        ):
            # ---------------- loads (single SWDGE sem lane) ----------------
            st_pad = big.tile([P, NCH, NS, RT, XP], F32)
            qt = big.tile([P, NCH, PCNT, NQ], F32)
            nc.gpsimd.dma_start(out=st_pad[:], in_=sin[:])
            nc.gpsimd.dma_start(out=qt[:], in_=qin[:])

            eps = big.tile([1, 1], F32)
            nc.vector.memset(eps[:], 1e-16)

            # ---------------- norms: ssq -> sqrt -> reciprocal -------------
            st_flat = st_pad.rearrange("p c s r x -> p c (s r x)")
            qt_flat = qt.rearrange("p c a q -> p c (a q)")

            n_sqrt = big.tile([1, SP_COLS], F32)   # ACT-written
            m_sqrt = big.tile([1, Q_COLS], F32)
            n_inv = big.tile([1, SP_COLS], F32)    # DVE-written
            m_inv = big.tile([1, Q_COLS], F32)

            for (flat, ncols, dst) in ((st_flat, SP_COLS, n_sqrt), (qt_flat, Q_COLS, m_sqrt)):
                for off, n in _ceil_blocks(ncols, NBLK):
                    ssq = psn.tile([1, NBLK], F32, tag="ssq")
                    for ch in range(NCH):
                        sq = sqp.tile([P, NBLK], F32, tag="sq")
                        nc.scalar.activation(
                            out=sq[:, :n], in_=flat[:, ch, off:off + n],
                            func=mybir.ActivationFunctionType.Square)
                        nc.tensor.matmul(ssq[:, :n], ones, sq[:, :n],
                                         start=(ch == 0), stop=(ch == NCH - 1))
                    nc.scalar.activation(
                        out=dst[:, off:off + n], in_=ssq[:, :n],
                        func=mybir.ActivationFunctionType.Sqrt, bias=eps[:])
            nc.vector.reciprocal(out=n_inv[:], in_=n_sqrt[:])
            nc.vector.reciprocal(out=m_inv[:], in_=m_sqrt[:])

            # ------------- broadcast / transpose via DRAM round-trip -------
            n_dram = dram.tile([1, SP_COLS], F32)
            m_dram = dram.tile([1, Q_COLS], F32)
            nc.gpsimd.dma_start(out=n_dram[:], in_=n_inv[:])
            nc.gpsimd.dma_start(out=m_dram[:], in_=m_inv[:])

            invb = big.tile([P, SP_COLS], F32)
            src = bass.AP(tensor=n_dram.tensor, offset=n_dram.offset,
                          ap=[[0, P], [1, SP_COLS]])
            nc.gpsimd.dma_start(out=invb[:], in_=src)

            # inv_q transposed to [q, p] so it can be a per-partition scalar
            invq_t = big.tile([NQ, PCNT], F32)
            srcq = bass.AP(tensor=m_dram.tensor, offset=m_dram.offset,
                           ap=[[1, NQ], [NQ, PCNT]])
            nc.gpsimd.dma_start(out=invq_t[:], in_=srcq)

            # ---------------- scale support in place (DVE) ------------------
            for ch in range(NCH):
                nc.vector.tensor_mul(st_flat[:, ch], st_flat[:, ch], invb[:])

            # ------------- absorb the SWDGE sem on PE (1-wait rule) ---------
            scratch = psd.tile([1, NBLK], F32, tag="scratch")
            nc.tensor.matmul(scratch[:], ones, qt_flat[:, 0, 0:NBLK],
                             start=True, stop=True)

            # ---------------- main windowed matmuls -------------------------
            SA = 13          # s-split: 13 + 12
            W2 = 7           # stage half-rows to bound SBUF
            for v in range(VR):
                for half in range(W // W2):
                    stage = stp.tile([NQ, NS, W2, KK], F32, tag="stage")
                    for xi in range(W2):
                        x = half * W2 + xi
                        pa = psa.tile([NQ, SA, 5, 5], F32, tag="pa")
                        pb = psb.tile([NQ, NS - SA, 5, 5], F32, tag="pb")
                        for ch in range(NCH):
                            lhsT = qt[:, ch, v * W + x, :]
                            nc.tensor.matmul(
                                pa[:], lhsT, st_pad[:, ch, :SA, v:v + 5, x:x + 5],
                                start=(ch == 0), stop=(ch == NCH - 1))
                            nc.tensor.matmul(
                                pb[:], lhsT, st_pad[:, ch, SA:, v:v + 5, x:x + 5],
                                start=(ch == 0), stop=(ch == NCH - 1))
                        sc = invq_t[:, v * W + x: v * W + x + 1]
                        nc.vector.tensor_scalar_mul(
                            stage[:, :SA, xi, :], pa.rearrange("q s a b -> q s (a b)"), sc)
                        nc.vector.tensor_scalar_mul(
                            stage[:, SA:, xi, :], pb.rearrange("q s a b -> q s (a b)"), sc)
                    p0 = v * W + half * W2
                    nc.gpsimd.dma_start(out=out[:, :, p0:p0 + W2, :], in_=stage[:])
    nc.compile()
    return nc


def _prep_inputs(support, query):
    """Host-side shard + layout prep (pure data movement, no FLOPs)."""
    support = np.ascontiguousarray(support, dtype=np.float32)
    query = np.ascontiguousarray(query, dtype=np.float32)

    # query -> (c_in, chunk, p, q); pad rows 14,15 with zeros
    q_t = query.reshape(NQ, NCH, P, H * W).transpose(2, 1, 3, 0)  # (128,5,196,75)
    q_pad = np.zeros((P, NCH, 16 * W, NQ), dtype=np.float32)
    q_pad[:, :, :H * W, :] = q_t

    # support -> (c_in, chunk, s, row_padded(19 = 2+14+3), x_padded(18))
    s_t = support.reshape(NS, NCH, P, H, W).transpose(2, 1, 0, 3, 4)  # (128,5,25,14,14)
    s_pad = np.zeros((P, NCH, NS, H + 5, XP), dtype=np.float32)
    s_pad[:, :, :, 2:2 + H, 2:2 + W] = s_t

    in_maps = []
    for core in range(NCORES):
        rb = ROW_BASE[core]
        if core < 6:
            qin = np.ascontiguousarray(q_pad[:, :, rb * W:(rb + VR) * W, :])
        else:
            qin = np.zeros((P, NCH, PCNT, NQ), dtype=np.float32)
            qin[:, :, :W, :] = q_pad[:, :, rb * W:(rb + 1) * W, :]
        sin = np.ascontiguousarray(s_pad[:, :, :, rb:rb + RT, :])
        in_maps.append({"qin": qin, "sin": sin})
    return in_maps


def _gather_output(results):
    parts = []
    for core in range(NCORES):
        o = results[core]["out"]          # (75, 25, 28, 25)
        parts.append(o[:, :, :ROW_CNT[core] * W, :])
    return np.concatenate(parts, axis=2)  # (75, 25, 196, 25)


def kernel(support, query, _trace=False):
    if "nc" not in _NC_CACHE:
        _NC_CACHE["nc"] = build_nc()
    nc = _NC_CACHE["nc"]
    in_maps = _prep_inputs(support, query)
    res = run_bass_kernel_spmd(nc, in_maps, core_ids=list(range(NCORES)),
                               trace=_trace)
    out = _gather_output(res.results)
    if _trace:
        kernel.last_result = res
    return out
